# revision 34
# baseline (speedup 1.0000x reference)
"""Trainium2 Bass kernel for nn_ConditionedConvolution2D, v8.

Reference computation:
    A  = P @ dense_w                      # [B, 3*3*C*C_OUT] per-sample conv kernels
    Wk = A.reshape(B, 3, 3, C, C_OUT)
    Y[b] = conv2d(X[b], Wk[b])            # SAME padding, stride 1, NHWC

Strategy (pure data parallel, 4 samples per core on 8 cores):
  - The core is HBM-bound: wall ~= startup + total_bytes/358GB/s + drain.
    So X ships as the MINIMAL float8_e3m4 "shifted triple" im2col slab
    (planes j=dw of row hp-1; 6.5MB/core — e3m4's 4 mantissa bits keep
    rel-l2 at 1.4e-2 against the 2e-2 gate; fp8-e4m3 or fp8 weights fail).
    Weights and output stay bf16.
  - The matmul stationary is still read as K=128 ([128, 128] fast-weight-
    load stays on the LDWEIGHTS-hidden path; K=96 stationaries expose
    ~100ns/row of weight-load): slab partitions 96..127 are memset to zero
    once per slab buffer and contribute 0 to every accumulation (w_sb rows
    96..127 are likewise zeroed so no NaN*0 can poison PSUM).
  - Hypernetwork: 96 matmuls (lhsT = host-permuted dense_w [128, 96] bf16,
    rhs = P^T [128, 4]) into one PSUM tile; one DVE copy permutes
    (g,b)->(b,g) casting f32->bf16 into the conv moving operand
    w_sb[(dw,ci), b*96 + (2-dh)*32 + co].
  - Conv: PSUM banks hold 16 output rows ([128 w, 16*32]); one matmul per
    slab row (moving [128, <=96]) writes row-chunks r = hp-2, hp-1, hp in
    one shot.  Pad rows are not shipped; boundary rows just receive fewer
    accumulations.
  - DMA ring split: all input traffic (hypernet weight chunks, then slabs)
    rides the SP HWDGE ring (nc.sync) in FIFO order; all output traffic
    rides the ACT HWDGE ring (nc.scalar).  Completed banks are cast to bf16
    (alternating DVE/ACT) into per-sample [128, 4096] staging, shipped
    256KB per quarter-sample; host transposes back to NHWC.
"""

import os
import sys

sys.path.insert(0, "/opt/trn_rl_repo")

import numpy as np
import ml_dtypes

import concourse.bacc as bacc
import concourse.mybir as mybir
import concourse.tile as tile
from concourse.bass_utils import run_bass_kernel_spmd

B, H, W, C = 32, 128, 128, 32
P_DIM = 128
KH = KW = 3
C_OUT = 32
N_CORES = 8
BPC = B // N_CORES          # samples per core
W2 = 132                    # padded row pitch
QK = KW * C                 # 96 shipped planes (dw, ci)
G = KH * C_OUT              # 96 weight-stream columns per sample (2-dh, co)
RPT = 16                    # output rows per PSUM tile (one full bank)

_NC_CACHE = {}
BF16 = ml_dtypes.bfloat16
E3M4 = ml_dtypes.float8_e3m4


def _build_nc():
    f32 = mybir.dt.float32
    bf16 = mybir.dt.bfloat16
    e3m4 = mybir.dt.float8e3
    nc = bacc.Bacc("TRN2", target_bir_lowering=False, debug=False,
                   num_devices=N_CORES)
    x_trip = nc.dram_tensor("x_trip", [BPC, QK, H * W2], e3m4,
                            kind="ExternalInput")
    p_t = nc.dram_tensor("p_t", [P_DIM, BPC], bf16, kind="ExternalInput")
    dw_t = nc.dram_tensor("dw_t", [P_DIM, G * QK], bf16,
                          kind="ExternalInput")
    y = nc.dram_tensor("y", [BPC, W, H * C_OUT], bf16, kind="ExternalOutput")

    with tile.TileContext(nc) as tc:
        with tc.tile_pool(name="const", bufs=1) as cpool, \
             tc.tile_pool(name="wsb", bufs=1) as wsb_pool, \
             tc.tile_pool(name="slab", bufs=1) as slab_pool, \
             tc.tile_pool(name="osb", bufs=4) as osb_pool:

            p_sb = cpool.tile([P_DIM, BPC], bf16, name="p_sb", tag="p_sb")
            nc.scalar.dma_start(out=p_sb[:], in_=p_t[:])
            dwsb = cpool.tile([P_DIM, G * QK], bf16, name="dwsb", tag="dwsb")
            NSPLIT = 5
            gsz = (G + NSPLIT - 1) // NSPLIT
            for k in range(NSPLIT):
                glo, ghi = min(k * gsz, G), min((k + 1) * gsz, G)
                nc.sync.dma_start(
                    out=dwsb[:, glo * QK:ghi * QK],
                    in_=dw_t[:, glo * QK:ghi * QK])

            # 3 manually-rotated slab buffers, zero planes pre-memset once
            pre_slabs = [slab_pool.tile([P_DIM, H * W2], e3m4,
                                        name=f"slab{i}", tag=f"slab{i}")
                         for i in range(3)]
            # zero partitions 96..127 of each buffer on DVE (idle until the
            # hypernet finishes); buffer 2's memset is emitted after the
            # w_sb permute-copy so it cannot delay the conv start
            nc.vector.memset(pre_slabs[0][QK:P_DIM, :], 0.0)
            nc.vector.memset(pre_slabs[1][QK:P_DIM, :], 0.0)

            def load_slab(b, nsplit):
                slab = pre_slabs[b % 3]
                rows = [H * k // nsplit for k in range(nsplit + 1)]
                for k in range(nsplit):
                    nc.sync.dma_start(
                        out=slab[0:QK, rows[k] * W2:rows[k + 1] * W2],
                        in_=x_trip[b][:, rows[k] * W2:rows[k + 1] * W2])
                return slab

            # w_sb[(dw,ci), b*G + (2-dh)*C_OUT + co] (bf16 stream operand)
            w_sb = wsb_pool.tile([P_DIM, BPC * G], bf16, name="w_sb",
                                 tag="w_sb")
            # rows 96..127 multiply the zeroed slab planes; keep them finite
            nc.gpsimd.memset(w_sb[QK:P_DIM, :], 0.0)

            # ---- Phase 0: hypernetwork  Wk = P @ dense_w (permuted) ----
            with tc.tile_pool(name="wps", bufs=1, space="PSUM") as wps_pool:
                wps = wps_pool.tile([QK, G * BPC], f32, name="wps", tag="wps")
                for g in range(G):      # g = (2-dh)*C_OUT + co
                    nc.tensor.matmul(
                        out=wps[:, g * BPC:(g + 1) * BPC],
                        lhsT=dwsb[:, g * QK:(g + 1) * QK],
                        rhs=p_sb[:],
                        start=True, stop=True,
                    )
                # permute (g, b) -> (b, g) while casting f32 -> bf16
                src = wps[:].rearrange("p (g b) -> p g b", b=BPC)
                dst = w_sb[0:QK, :].rearrange("p (b g) -> p g b", g=G)
                nc.vector.tensor_copy(out=dst, in_=src)
                nc.vector.memset(pre_slabs[2][QK:P_DIM, :], 0.0)

            # ---- Phase 1: per-sample conv ----
            with tc.tile_pool(name="acc", bufs=6, space="PSUM") as acc_pool:
                for b in range(BPC):
                    slab = load_slab(b, 3 if b == 0 else 2)
                    osb = osb_pool.tile([W, H * C_OUT], bf16, name="osb",
                                        tag="osb")

                    tiles = {}      # t -> psum AP [W, RPT*C_OUT]
                    for hp in range(1, H + 1):
                        lhsT = slab[:, (hp - 1) * W2: (hp - 1) * W2 + W]
                        rows = [r for r in (hp - 2, hp - 1, hp)
                                if 0 <= r < H]
                        groups = []
                        for r in rows:
                            t = r // RPT
                            if groups and groups[-1][0] == t:
                                groups[-1][1].append(r)
                            else:
                                groups.append((t, [r]))
                        for t, rs in groups:
                            start = t not in tiles
                            if start:
                                tiles[t] = acc_pool.tile(
                                    [W, RPT * C_OUT], f32, name="acc",
                                    tag="acc")
                            r_lo = rs[0]
                            c_lo = r_lo % RPT
                            w_lo = 2 - (hp - r_lo)
                            last = hp == min(t * RPT + RPT + 1, H)
                            nc.tensor.matmul(
                                out=tiles[t][:, c_lo * C_OUT:
                                             (c_lo + len(rs)) * C_OUT],
                                lhsT=lhsT,
                                rhs=w_sb[:, b * G + w_lo * C_OUT:
                                         b * G + (w_lo + len(rs)) * C_OUT],
                                start=start, stop=last,
                                skip_group_check=True,
                            )
                        for t in list(tiles):
                            if hp == min(t * RPT + RPT + 1, H):
                                src2 = tiles.pop(t)
                                dst2 = osb[:, t * RPT * C_OUT:
                                           (t + 1) * RPT * C_OUT]
                                if t % 2 == 0:
                                    nc.vector.tensor_copy(out=dst2,
                                                          in_=src2[:])
                                else:
                                    nc.scalar.copy(out=dst2, in_=src2[:])
                                if t % 2 == 1:
                                    qf = t // 2
                                    nc.scalar.dma_start(
                                        out=y[b][:, qf * 2 * RPT * C_OUT:
                                                 (qf + 1) * 2 * RPT * C_OUT],
                                        in_=osb[:, qf * 2 * RPT * C_OUT:
                                                (qf + 1) * 2 * RPT * C_OUT])
    nc.finalize()
    return nc


def _get_nc():
    if "nc" not in _NC_CACHE:
        _NC_CACHE["nc"] = _build_nc()
    return _NC_CACHE["nc"]


def _prep_inputs(X, P, dense_w):
    Xb = np.ascontiguousarray(X.transpose(0, 3, 1, 2))   # [B,C,H,W] f32
    # X_trip[b, dw*32+ci, hp-1, wp] = X[b, hp-1, wp+dw-1, ci] (0 outside)
    X_trip = np.zeros((B, QK, H, W2), dtype=E3M4)
    for dw in range(KW):
        lo = max(0, 1 - dw)
        hi = W - dw
        src_lo = lo + dw - 1
        X_trip[:, dw * C:(dw + 1) * C, :, lo:hi + 1] = \
            Xb[:, :, :, src_lo:W].astype(E3M4)
    X_trip = X_trip.reshape(B, QK, H * W2)

    # dense_w columns j = ((dh*3+dw)*C+ci)*C_OUT+co -> (2-dh, co, dw, ci)
    dwp = np.ascontiguousarray(
        dense_w.reshape(P_DIM, KH, KW, C, C_OUT)[:, ::-1]
        .transpose(0, 1, 4, 2, 3)
        .reshape(P_DIM, -1)
    ).astype(BF16)

    in_maps = []
    for c in range(N_CORES):
        sl = slice(c * BPC, (c + 1) * BPC)
        in_maps.append({
            "x_trip": np.ascontiguousarray(X_trip[sl]),
            "p_t": np.ascontiguousarray(P[sl].T).astype(BF16),
            "dw_t": dwp,
        })
    return in_maps


def _run(X, P, dense_w, **spmd_kwargs):
    nc = _get_nc()
    in_maps = _prep_inputs(X, P, dense_w)
    res = run_bass_kernel_spmd(nc, in_maps, core_ids=list(range(N_CORES)),
                               **spmd_kwargs)
    outs = []
    for c in range(N_CORES):
        yv = res.results[c]["y"].astype(np.float32)
        yv = yv.reshape(BPC, W, H, C_OUT)
        outs.append(yv.transpose(0, 2, 1, 3))        # -> [b, h, w, co]
    Y = np.ascontiguousarray(np.concatenate(outs, axis=0), dtype=np.float32)
    return Y, res


def kernel(X, P, dense_w):
    Y, _ = _run(np.asarray(X), np.asarray(P), np.asarray(dense_w))
    return Y


# revision 36
# speedup vs baseline: 1.5353x; 1.5353x over previous
"""Trainium2 Bass kernel for nn_ConditionedConvolution2D, v5 (QUAD).

Reference computation:
    A  = P @ dense_w                      # [B, 3*3*C*C_OUT] per-sample conv kernels
    Wk = A.reshape(B, 3, 3, C, C_OUT)
    Y[b] = conv2d(X[b], Wk[b])            # SAME padding, stride 1, NHWC

Strategy (pure data parallel, 4 samples per core on 8 cores):
  - Host pre-lays X as a float8_e3m4 "QUAD" im2col slab with K=128:
    planes j=0..2 are the w-shifted triple of row s-1 (dw = j), plane j=3 is
    row s at dw=0.  Each slab row s in [0,128] yields a full-width stationary
    lhsT [128=(j,ci), 128=w] in one AP.  vs the 96-partition triple this (a)
    uses all 16 SDMA engines (96-wide transfers leave the odd engines idle,
    so the extra plane ships in otherwise-dead engine time), and (b) makes
    LDWEIGHTS a full 128x128 fast-weight-load.  e3m4 (4 mantissa bits,
    ~1.4e-2 rel-l2, gate 2e-2) halves X bytes vs bf16; weights and output
    stay bf16 (fp8 weights would fail the gate).
  - Per (s, output row r): weight block beta = 2-(s-r) selects (dh, dw)
    pairs per plane: beta=2: {W00,W01,W02,W10}, beta=1: {0,W11,W12,W20},
    beta=0: {0,W21,W22,0} — each of the 9 taps counted exactly once; the
    boundary rows lose only pad-zero contributions.  The zero blocks are
    baked into the host-permuted hypernetwork weight so the 96 hypernet
    matmuls emit the conv moving operand layout directly.
  - Conv: PSUM accumulators hold 16 output rows ([128 w, 16*32]); one
    mixed-dtype matmul (stationary e3m4, moving bf16 [128, <=96]) per slab
    row writes row-chunks r = s-2, s-1, s in one shot.  Completed banks are
    cast to bf16 (alternating DVE/ACT) into a per-sample [128, 4096] staging
    tile, shipped 512KB per half-sample; host transposes back to NHWC.
"""

import os
import sys

sys.path.insert(0, "/opt/trn_rl_repo")

import numpy as np
import ml_dtypes

import concourse.bacc as bacc
import concourse.mybir as mybir
import concourse.tile as tile
from concourse.bass_utils import run_bass_kernel_spmd

B, H, W, C = 32, 128, 128, 32
P_DIM = 128
KH = KW = 3
C_OUT = 32
N_CORES = 8
BPC = B // N_CORES          # samples per core
W2 = 132                    # padded row pitch
S = H + 1                   # slab rows (s=0 carries the j=3-only edge row)
G = KH * C_OUT              # 96 weight-stream columns per sample (beta, co)
RPT = 16                    # output rows per PSUM tile (one full bank)

# hypernet stationary widths: block beta taps exactly planes 0..BWID/32-1
BWID = {0: 64, 1: 96, 2: 128}    # beta -> stationary width (cols)
DWTOT = 32 * (64 + 96 + 128)     # 9216 total dwq columns

# Plane layout (chosen so lower-beta blocks tap a prefix of planes, letting
# their hypernet stationaries shrink to base-0 widths 64/96/128 with no pad):
#   j=0 <- (row s-1, dw=1), j=1 <- (row s-1, dw=2),
#   j=2 <- (row s,   dw=0), j=3 <- (row s-1, dw=0)
# T[beta][j] = (dh, dw) of the tap carried by plane j in weight block beta
T = {2: {3: (0, 0), 0: (0, 1), 1: (0, 2), 2: (1, 0)},
     1: {0: (1, 1), 1: (1, 2), 2: (2, 0)},
     0: {0: (2, 1), 1: (2, 2)}}

_NC_CACHE = {}
BF16 = ml_dtypes.bfloat16
E3M4 = ml_dtypes.float8_e3m4


def _build_nc():
    f32 = mybir.dt.float32
    bf16 = mybir.dt.bfloat16
    e3m4 = mybir.dt.float8e3
    nc = bacc.Bacc("TRN2", target_bir_lowering=False, debug=False,
                   num_devices=N_CORES)
    x_quad = nc.dram_tensor("x_quad", [BPC, P_DIM, S * W2], e3m4,
                            kind="ExternalInput")
    p_t = nc.dram_tensor("p_t", [P_DIM, BPC], bf16, kind="ExternalInput")
    dw_t = nc.dram_tensor("dw_t", [P_DIM, DWTOT], bf16,
                          kind="ExternalInput")
    y = nc.dram_tensor("y", [BPC, W, H * C_OUT], bf16, kind="ExternalOutput")

    with tile.TileContext(nc) as tc:
        with tc.tile_pool(name="const", bufs=1) as cpool, \
             tc.tile_pool(name="wsb", bufs=1) as wsb_pool, \
             tc.tile_pool(name="slab", bufs=3) as slab_pool, \
             tc.tile_pool(name="osb", bufs=4) as osb_pool:

            # ---- Phase 0: hypernetwork  Wk = P @ dense_w (permuted) ----
            # DMA ring split: ALL input traffic (hypernet weight chunks, then
            # slabs) rides the SP HWDGE ring (nc.sync) in FIFO order, so the
            # slab stream is never bandwidth-shared against the weight
            # stream once the hypernet is fed; ALL output traffic rides the
            # ACT HWDGE ring (nc.scalar) and overlaps the input stream at
            # the SDMA round-robin level.
            p_sb = cpool.tile([P_DIM, BPC], bf16, name="p_sb", tag="p_sb")
            nc.scalar.dma_start(out=p_sb[:], in_=p_t[:])
            dwsb = cpool.tile([P_DIM, DWTOT], bf16, name="dwsb", tag="dwsb")
            goff = [0]
            for g in range(G):
                goff.append(goff[-1] + BWID[g // 32])

            slabs = {}

            def load_slab(b, nsplit):
                slab = slab_pool.tile([P_DIM, S * W2], e3m4, name="slab",
                                      tag="slab")
                rows = [S * k // nsplit for k in range(nsplit + 1)]
                for k in range(nsplit):
                    nc.sync.dma_start(
                        out=slab[:, rows[k] * W2:rows[k + 1] * W2],
                        in_=x_quad[b][:, rows[k] * W2:rows[k + 1] * W2])
                slabs[b] = slab

            # input-ring FIFO order: dwq chunks first (they gate the
            # hypernet, which gates everything), then slabs stream
            NSPLIT = 5
            gsz = (G + NSPLIT - 1) // NSPLIT
            for k in range(NSPLIT):
                glo, ghi = k * gsz, min((k + 1) * gsz, G)
                nc.sync.dma_start(
                    out=dwsb[:, goff[glo]:goff[ghi]],
                    in_=dw_t[:, goff[glo]:goff[ghi]])

            # w_sb[(j,ci), b*G + beta*32 + co] (bf16 stream operand)
            w_sb = wsb_pool.tile([P_DIM, BPC * G], bf16, name="w_sb",
                                 tag="w_sb")

            with tc.tile_pool(name="wps", bufs=1, space="PSUM") as wps_pool:
                wps = wps_pool.tile([P_DIM, G * BPC], f32, name="wps",
                                    tag="wps")
                for g in range(G):      # g = beta*32 + co
                    wd = BWID[g // 32]
                    nc.tensor.matmul(
                        out=wps[0:wd, g * BPC:(g + 1) * BPC],
                        lhsT=dwsb[:, goff[g]:goff[g + 1]],
                        rhs=p_sb[:],
                        start=True, stop=True,
                    )
                # permute (g, b) -> (b, g) while casting f32 -> bf16
                src = wps[:].rearrange("p (g b) -> p g b", b=BPC)
                dst = w_sb[:].rearrange("p (b g) -> p g b", g=G)
                nc.vector.tensor_copy(out=dst, in_=src)
                # blocks written by no matmul hold PSUM junk: zero them
                # (beta=0: planes 2,3 -> cols 0..31; beta=1: plane 3 -> 32..63)
                wv = w_sb[:].rearrange("p (b g) -> p b g", g=G)
                nc.vector.memset(wv[64:128, :, 0:32], 0.0)
                nc.vector.memset(wv[96:128, :, 32:64], 0.0)

            # ---- Phase 1: per-sample conv ----
            with tc.tile_pool(name="acc", bufs=6, space="PSUM") as acc_pool:
                for b in range(BPC):
                    if b not in slabs:
                        load_slab(b, 2)
                    slab = slabs.pop(b)

                    osb = osb_pool.tile([W, H * C_OUT], bf16, name="osb",
                                        tag="osb")

                    tiles = {}      # t -> psum AP [W, RPT*C_OUT]
                    for s in range(S):
                        lhsT = slab[:, s * W2: s * W2 + W]
                        rows = [r for r in (s - 2, s - 1, s) if 0 <= r < H]
                        groups = []
                        for r in rows:
                            t = r // RPT
                            if groups and groups[-1][0] == t:
                                groups[-1][1].append(r)
                            else:
                                groups.append((t, [r]))
                        for t, rs in groups:
                            start = t not in tiles
                            if start:
                                tiles[t] = acc_pool.tile(
                                    [W, RPT * C_OUT], f32, name="acc",
                                    tag="acc")
                            r_lo = rs[0]
                            c_lo = r_lo % RPT
                            w_lo = 2 - (s - r_lo)
                            last = s == min(t * RPT + RPT + 1, H)
                            nc.tensor.matmul(
                                out=tiles[t][:, c_lo * C_OUT:
                                             (c_lo + len(rs)) * C_OUT],
                                lhsT=lhsT,
                                rhs=w_sb[:, b * G + w_lo * C_OUT:
                                         b * G + (w_lo + len(rs)) * C_OUT],
                                start=start, stop=last,
                                skip_group_check=True,
                            )
                        for t in list(tiles):
                            if s == min(t * RPT + RPT + 1, H):
                                src2 = tiles.pop(t)
                                dst2 = osb[:, t * RPT * C_OUT:
                                           (t + 1) * RPT * C_OUT]
                                if t % 2 == 0:
                                    nc.vector.tensor_copy(out=dst2,
                                                          in_=src2[:])
                                else:
                                    nc.scalar.copy(out=dst2, in_=src2[:])
                                if t == 3 or t == 7:
                                    hf = (t - 3) // 4
                                    nc.scalar.dma_start(
                                        out=y[b][:, hf * 4 * RPT * C_OUT:
                                                 (hf + 1) * 4 * RPT * C_OUT],
                                        in_=osb[:, hf * 4 * RPT * C_OUT:
                                                (hf + 1) * 4 * RPT * C_OUT])
    nc.finalize()
    return nc


def _get_nc():
    if "nc" not in _NC_CACHE:
        _NC_CACHE["nc"] = _build_nc()
    return _NC_CACHE["nc"]


def _prep_inputs(X, P, dense_w):
    Xb = np.ascontiguousarray(X.transpose(0, 3, 1, 2))   # [B,C,H,W] f32
    # x_quad[b, j*32+ci, s, wp] per the plane layout in T's comment:
    #   j in {0,1,3}: X[b, s-1, wp+dw-1, ci] with dw = {0:1, 1:2, 3:0}[j]
    #   j=2:          X[b, s,   wp-1,    ci] (s=128 -> 0)
    Xq = Xb.astype(E3M4)
    x_quad = np.zeros((B, P_DIM, S, W2), dtype=E3M4)
    for j, dw in {0: 1, 1: 2, 3: 0}.items():
        lo = max(0, 1 - dw)
        hi = W - dw
        src_lo = lo + dw - 1
        x_quad[:, j * C:(j + 1) * C, 1:S, lo:hi + 1] = Xq[:, :, :, src_lo:W]
    x_quad[:, 2 * C:3 * C, 0:H, 1:1 + W] = Xq
    x_quad = x_quad.reshape(B, P_DIM, S * W2)

    # hypernet weight, variable width per block:
    # dwq[p, goff[g] + j*32+ci] = dense_w[p, (dh,dw,ci,co)] for
    # (dh, dw) = T[g//32][j], else 0   (g = beta*32 + co)
    dwr = dense_w.reshape(P_DIM, KH, KW, C, C_OUT)
    dwq = np.zeros((P_DIM, DWTOT), dtype=BF16)
    goff = [0]
    for g in range(G):
        goff.append(goff[-1] + BWID[g // 32])
    for g in range(G):
        beta, co = g // 32, g % 32
        for j, (dh, dw) in T[beta].items():
            if j * 32 < BWID[beta]:
                dwq[:, goff[g] + j * 32:goff[g] + (j + 1) * 32] = \
                    dwr[:, dh, dw, :, co].astype(BF16)

    in_maps = []
    for c in range(N_CORES):
        sl = slice(c * BPC, (c + 1) * BPC)
        in_maps.append({
            "x_quad": np.ascontiguousarray(x_quad[sl]),
            "p_t": np.ascontiguousarray(P[sl].T).astype(BF16),
            "dw_t": dwq,
        })
    return in_maps


def _run(X, P, dense_w, **spmd_kwargs):
    nc = _get_nc()
    in_maps = _prep_inputs(X, P, dense_w)
    res = run_bass_kernel_spmd(nc, in_maps, core_ids=list(range(N_CORES)),
                               **spmd_kwargs)
    outs = []
    for c in range(N_CORES):
        yv = res.results[c]["y"].astype(np.float32)
        yv = yv.reshape(BPC, W, H, C_OUT)
        outs.append(yv.transpose(0, 2, 1, 3))        # -> [b, h, w, co]
    Y = np.ascontiguousarray(np.concatenate(outs, axis=0), dtype=np.float32)
    return Y, res


def kernel(X, P, dense_w):
    Y, _ = _run(np.asarray(X), np.asarray(P), np.asarray(dense_w))
    return Y
